# revision 12
# baseline (speedup 1.0000x reference)
"""Multi-head causal attention (B=2, S=2048, D=2048, H=16, HD=128) on 8 TRN2
NeuronCores.

Sharding: data-parallel over batch (2 groups of 4 cores) x tensor-parallel
over heads (4 heads per core).  Each core computes q/k/v projections for its
512 columns (4 heads), causal attention for those heads, and a partial
(row-sharded-contraction) wo product.  The 4 partial outputs per batch are
summed on the host (the "all-reduce after wo" of the sharding hint).

Everything on-chip is computed in transposed orientation:
  xT [d, s] (host pre-transposed), qT/kT [j, s], scores^T [t, s], out^T [j2, s]
so that every matmul contraction lands on the partition axis with zero
on-chip transposes.  Softmax uses exp without max-subtraction (scores are
O(5), exact for fp32) with denominators from a ones-vector matmul; causal
masking is applied post-exp via affine_select (exact zeros, matching the
reference's exp(-1e9) == 0 underflow).
"""

import numpy as np

import concourse.bass as bass
import concourse.tile as tile
from concourse import bacc, mybir
from concourse.bass_utils import run_bass_kernel_spmd

B, S, D = 2, 2048, 2048
H, HD = 16, 128
P = 128
JL = 512          # local q/k/v columns per core (4 heads)
NH = 4            # heads per core
CHUNK = 512       # s-chunk
NCH = S // CHUNK  # 4
DT = D // P       # 16 d-tiles
NT = S // P       # 16 t-tiles
SCALE = 1.0 / float(np.sqrt(HD))

F32 = mybir.dt.float32
F32R = mybir.dt.float32r


def build_kernel():
    nc = bacc.Bacc("TRN2", target_bir_lowering=False, debug=False, num_devices=8)
    xT = nc.dram_tensor("xT", [D, S], F32R, kind="ExternalInput").ap()
    wqT = nc.dram_tensor("wqT", [D, JL], F32R, kind="ExternalInput").ap()
    wkT = nc.dram_tensor("wkT", [D, JL], F32R, kind="ExternalInput").ap()
    wvT = nc.dram_tensor("wvT", [D, JL], F32R, kind="ExternalInput").ap()
    woT = nc.dram_tensor("woT", [JL, D], F32R, kind="ExternalInput").ap()
    outT = nc.dram_tensor("outT", [D, S], F32, kind="ExternalOutput").ap()

    with tile.TileContext(nc) as tc:
        with (
            tc.tile_pool(name="persist", bufs=1) as persist,
            tc.tile_pool(name="xt", bufs=1) as xt_pool,
            tc.tile_pool(name="wst", bufs=6) as wst_pool,
            tc.tile_pool(name="qt", bufs=2) as qt_pool,
            tc.tile_pool(name="exp", bufs=6) as exp_pool,
            tc.tile_pool(name="ot", bufs=4) as ot_pool,
            tc.tile_pool(name="small", bufs=2) as small_pool,
            tc.tile_pool(name="osb", bufs=3) as osb_pool,
            tc.tile_pool(name="ps_main", bufs=1, space="PSUM") as ps_main,
            tc.tile_pool(name="ps_s", bufs=3, space="PSUM") as ps_s,
            tc.tile_pool(name="ps_rs", bufs=1, space="PSUM") as ps_rs,
        ):
            master = persist.tile([P, 896], F32, name="master")
            nc.gpsimd.memset(master[:], 1.0)
            # master[p, u] = 1.0 iff u - p - 384 >= 0 else 0.0
            nc.gpsimd.affine_select(
                out=master[:], in_=master[:], pattern=[[1, 896]],
                compare_op=mybir.AluOpType.is_ge, fill=0.0,
                base=-384, channel_multiplier=-1,
            )
            ones_f = persist.tile([P, 1], F32, name="ones_f")
            nc.vector.memset(ones_f[:], 1.0)
            ones = persist.tile([P, 1], F32R, name="ones")
            nc.vector.tensor_copy(ones[:], ones_f[:])

            # persistent K^T tiles [e(head), t] and V tiles [t, j]
            kT_t = [persist.tile([P, S], F32R, name=f"kT{h}") for h in range(NH)]
            v_t = [persist.tile([P, JL], F32R, name=f"v{t}") for t in range(NT)]
            woT_t = [persist.tile([P, D], F32R, name=f"woT{h}") for h in range(NH)]
            for h in range(NH):
                nc.scalar.dma_start(out=woT_t[h][:], in_=woT[h * P:(h + 1) * P, :])

            for c in range(NCH):
                s0 = c * CHUNK
                ssl = slice(s0, s0 + CHUNK)

                # ---- k projection (xT chunk tiles loaded inline) ----
                ps_k = [ps_main.tile([P, CHUNK], F32, name=f"psk{j}", tag=f"pm{j}")
                        for j in range(4)]
                xt = []
                for d in range(DT):
                    t_ = xt_pool.tile([P, CHUNK], F32R, name=f"xt{d}", tag=f"xt{d}")
                    nc.sync.dma_start(out=t_[:], in_=xT[d * P:(d + 1) * P, ssl])
                    xt.append(t_)
                    wk_d = wst_pool.tile([P, JL], F32R, name=f"wk{d}", tag="wst")
                    nc.sync.dma_start(out=wk_d[:], in_=wkT[d * P:(d + 1) * P, :])
                    for j in range(4):
                        nc.tensor.matmul(
                            ps_k[j][:], wk_d[:, j * P:(j + 1) * P], xt[d][:],
                            start=(d == 0), stop=(d == DT - 1),
                            skip_group_check=True,
                        )
                for j in range(4):
                    nc.vector.tensor_copy(kT_t[j][:, ssl], ps_k[j][:])

                # ---- q projection: qT[j, s-chunk] ----
                ps_q = [ps_main.tile([P, CHUNK], F32, name=f"psq{j}", tag=f"pm{j}")
                        for j in range(4)]
                for d in range(DT):
                    wq_d = wst_pool.tile([P, JL], F32R, name=f"wq{d}", tag="wst")
                    nc.sync.dma_start(out=wq_d[:], in_=wqT[d * P:(d + 1) * P, :])
                    for j in range(4):
                        nc.tensor.matmul(
                            ps_q[j][:], wq_d[:, j * P:(j + 1) * P], xt[d][:],
                            start=(d == 0), stop=(d == DT - 1),
                            skip_group_check=True,
                        )
                qt = []
                for j in range(4):
                    t_ = qt_pool.tile([P, CHUNK], F32R, name=f"qt{j}", tag=f"qt{j}")
                    nc.vector.tensor_copy(t_[:], ps_q[j][:])
                    qt.append(t_)

                # ---- v projection: v[t, j] for the 4 new t-tiles ----
                ps_v = [ps_main.tile([P, CHUNK], F32, name=f"psv{i}", tag=f"pm{i}")
                        for i in range(4)]
                for d in range(DT):
                    wv_d = wst_pool.tile([P, JL], F32R, name=f"wv{d}", tag="wst")
                    nc.sync.dma_start(out=wv_d[:], in_=wvT[d * P:(d + 1) * P, :])
                    for i in range(4):
                        nc.tensor.matmul(
                            ps_v[i][:], xt[d][:, i * P:(i + 1) * P], wv_d[:],
                            start=(d == 0), stop=(d == DT - 1),
                            skip_group_check=True,
                        )
                for i in range(4):
                    nc.vector.tensor_copy(v_t[4 * c + i][:], ps_v[i][:])

                # ---- attention for each head over t-tiles 0..4c+3 ----
                ots = []
                for h in range(NH):
                    T = 4 * c + 4
                    rs_acc = ps_rs.tile([1, CHUNK], F32, name="rsacc", tag="rs")
                    o_acc = ps_main.tile([P, CHUNK], F32, name="oacc", tag=f"pm{h}")
                    exps = [None] * T

                    def emit_b(t):
                        nc.tensor.matmul(
                            rs_acc[:], ones[:], exps[t][:],
                            start=(t == 0), stop=(t == T - 1),
                            skip_group_check=True,
                        )
                        nc.tensor.matmul(
                            o_acc[:], v_t[t][:, h * P:(h + 1) * P], exps[t][:],
                            start=(t == 0), stop=(t == T - 1),
                            skip_group_check=True,
                        )

                    ngroups = T // 4
                    for g in range(ngroups):
                        for i in range(4):
                            t = 4 * g + i
                            ps = ps_s.tile([P, CHUNK], F32, name="pss", tag="ss")
                            nc.tensor.matmul(
                                ps[:], kT_t[h][:, t * P:(t + 1) * P], qt[h][:],
                                start=True, stop=True, skip_group_check=True,
                            )
                            e = exp_pool.tile([P, CHUNK], F32R, name="exp", tag="exp")
                            nc.scalar.activation(
                                e[:], ps[:], mybir.ActivationFunctionType.Exp,
                                scale=SCALE,
                            )
                            if t >= 4 * c:
                                off = 384 + s0 - t * P
                                nc.vector.tensor_mul(
                                    e[:], e[:], master[:, off:off + CHUNK])
                            exps[t] = e
                        if g >= 1:
                            for i in range(4):
                                emit_b(4 * (g - 1) + i)
                    for i in range(4):
                        emit_b(4 * (ngroups - 1) + i)

                    # normalize: reciprocal (PSUM->SBUF), broadcast, multiply
                    rs_sb = small_pool.tile([1, CHUNK], F32, name="rssb", tag="rssb")
                    nc.vector.reciprocal_approx_fast(out=rs_sb[:], in_=rs_acc[:])
                    rb = small_pool.tile([P, CHUNK], F32, name="rb", tag="rb")
                    nc.gpsimd.partition_broadcast(rb[:], rs_sb[:])
                    ot = ot_pool.tile([P, CHUNK], F32R, name="ot", tag="ot")
                    nc.vector.tensor_mul(ot[:], o_acc[:], rb[:])
                    ots.append(ot)

                # ---- wo partial: outT[j2, s-chunk] = sum_h woT_h^T @ ot_h ----
                for j2 in range(DT):
                    pw = ps_s.tile([P, CHUNK], F32, name="pw", tag="ss")
                    for h in range(NH):
                        nc.tensor.matmul(
                            pw[:], woT_t[h][:, j2 * P:(j2 + 1) * P], ots[h][:],
                            start=(h == 0), stop=(h == NH - 1),
                            skip_group_check=True,
                        )
                    ob = osb_pool.tile([P, CHUNK], F32, name="ob", tag="ob")
                    nc.vector.tensor_copy(ob[:], pw[:])
                    nc.scalar.dma_start(out=outT[j2 * P:(j2 + 1) * P, ssl],
                                        in_=ob[:])

    nc.compile()
    return nc


_NC_CACHE = None


def _get_nc():
    global _NC_CACHE
    if _NC_CACHE is None:
        _NC_CACHE = build_kernel()
    return _NC_CACHE


def make_in_maps(x, wq, wk, wv, wo):
    in_maps = []
    for core in range(8):
        b, g = core // 4, core % 4
        j0 = g * JL
        in_maps.append({
            "xT": np.ascontiguousarray(x[b].T).astype(np.float32, copy=False),
            "wqT": np.ascontiguousarray(wq[j0:j0 + JL, :].T),
            "wkT": np.ascontiguousarray(wk[j0:j0 + JL, :].T),
            "wvT": np.ascontiguousarray(wv[j0:j0 + JL, :].T),
            "woT": np.ascontiguousarray(wo[:, j0:j0 + JL].T),
        })
    return in_maps


def kernel(x, freqs_complex=None, mask=None, wq=None, wk=None, wv=None, wo=None,
           **_unused):
    x = np.asarray(x, dtype=np.float32)
    wq = np.asarray(wq, dtype=np.float32)
    wk = np.asarray(wk, dtype=np.float32)
    wv = np.asarray(wv, dtype=np.float32)
    wo = np.asarray(wo, dtype=np.float32)

    nc = _get_nc()
    in_maps = make_in_maps(x, wq, wk, wv, wo)
    res = run_bass_kernel_spmd(nc, in_maps, list(range(8)))

    out = np.zeros((B, S, D), dtype=np.float32)
    for core in range(8):
        out[core // 4] += res.results[core]["outT"].T
    return out


# revision 13
# speedup vs baseline: 1.0641x; 1.0641x over previous
"""Multi-head causal attention (B=2, S=2048, D=2048, H=16, HD=128) on 8 TRN2
NeuronCores.

Sharding: data-parallel over batch (2 groups of 4 cores) x tensor-parallel
over heads (4 heads per core).  Each core computes q/k/v projections for its
512 columns (4 heads), causal attention for those heads, and a partial
(row-sharded-contraction) wo product.  The 4 partial outputs per batch are
summed on the host (the "all-reduce after wo" of the sharding hint).

Everything on-chip is computed in transposed orientation:
  xT [d, s] (host pre-transposed), qT/kT [j, s], scores^T [t, s], out^T [j2, s]
so that every matmul contraction lands on the partition axis with zero
on-chip transposes.  Softmax uses exp without max-subtraction (scores are
O(5), exact for fp32) with denominators from a ones-vector matmul; causal
masking is applied post-exp via affine_select (exact zeros, matching the
reference's exp(-1e9) == 0 underflow).
"""

import numpy as np

import concourse.bass as bass
import concourse.tile as tile
from concourse import bacc, mybir
from concourse.bass_utils import run_bass_kernel_spmd

B, S, D = 2, 2048, 2048
H, HD = 16, 128
P = 128
JL = 512          # local q/k/v columns per core (4 heads)
NH = 4            # heads per core
CHUNK = 512       # s-chunk
NCH = S // CHUNK  # 4
DT = D // P       # 16 d-tiles
NT = S // P       # 16 t-tiles
SCALE = 1.0 / float(np.sqrt(HD))

F32 = mybir.dt.float32
F32R = mybir.dt.float32r


def build_kernel():
    nc = bacc.Bacc("TRN2", target_bir_lowering=False, debug=False, num_devices=8)
    xT = nc.dram_tensor("xT", [D, S], F32R, kind="ExternalInput").ap()
    wqT = nc.dram_tensor("wqT", [D, JL], F32R, kind="ExternalInput").ap()
    wkT = nc.dram_tensor("wkT", [D, JL], F32R, kind="ExternalInput").ap()
    wvT = nc.dram_tensor("wvT", [D, JL], F32R, kind="ExternalInput").ap()
    woT = nc.dram_tensor("woT", [JL, D], F32R, kind="ExternalInput").ap()
    outT = nc.dram_tensor("outT", [D, S], F32, kind="ExternalOutput").ap()

    with tile.TileContext(nc) as tc:
        with (
            tc.tile_pool(name="persist", bufs=1) as persist,
            tc.tile_pool(name="xt", bufs=1) as xt_pool,
            tc.tile_pool(name="wst", bufs=6) as wst_pool,
            tc.tile_pool(name="qt", bufs=2) as qt_pool,
            tc.tile_pool(name="exp", bufs=6) as exp_pool,
            tc.tile_pool(name="ot", bufs=4) as ot_pool,
            tc.tile_pool(name="small", bufs=2) as small_pool,
            tc.tile_pool(name="osb", bufs=3) as osb_pool,
            tc.tile_pool(name="ps_main", bufs=1, space="PSUM") as ps_main,
            tc.tile_pool(name="ps_s", bufs=3, space="PSUM") as ps_s,
            tc.tile_pool(name="ps_rs", bufs=1, space="PSUM") as ps_rs,
        ):
            master = persist.tile([P, 896], F32, name="master")
            nc.gpsimd.memset(master[:], 1.0)
            # master[p, u] = 1.0 iff u - p - 384 >= 0 else 0.0
            nc.gpsimd.affine_select(
                out=master[:], in_=master[:], pattern=[[1, 896]],
                compare_op=mybir.AluOpType.is_ge, fill=0.0,
                base=-384, channel_multiplier=-1,
            )
            ones_f = persist.tile([P, 1], F32, name="ones_f")
            nc.vector.memset(ones_f[:], 1.0)
            ones = persist.tile([P, 1], F32R, name="ones")
            nc.vector.tensor_copy(ones[:], ones_f[:])

            # persistent K^T tiles [e(head), t] and V tiles [t, j]
            kT_t = [persist.tile([P, S], F32R, name=f"kT{h}") for h in range(NH)]
            v_t = [persist.tile([P, JL], F32R, name=f"v{t}") for t in range(NT)]
            woT_t = [persist.tile([P, D], F32R, name=f"woT{h}") for h in range(NH)]
            for h in range(NH):
                nc.scalar.dma_start(out=woT_t[h][:], in_=woT[h * P:(h + 1) * P, :])

            for c in range(NCH):
                s0 = c * CHUNK
                ssl = slice(s0, s0 + CHUNK)

                # ---- k projection (xT chunk tiles loaded inline) ----
                ps_k = [ps_main.tile([P, CHUNK], F32, name=f"psk{j}", tag=f"pm{j}")
                        for j in range(4)]
                xt = []
                for d in range(DT):
                    t_ = xt_pool.tile([P, CHUNK], F32R, name=f"xt{d}", tag=f"xt{d}")
                    nc.sync.dma_start(out=t_[:], in_=xT[d * P:(d + 1) * P, ssl])
                    xt.append(t_)
                    wk_d = wst_pool.tile([P, JL], F32R, name=f"wk{d}", tag="wst")
                    nc.sync.dma_start(out=wk_d[:], in_=wkT[d * P:(d + 1) * P, :])
                    for j in range(4):
                        nc.tensor.matmul(
                            ps_k[j][:], wk_d[:, j * P:(j + 1) * P], xt[d][:],
                            start=(d == 0), stop=(d == DT - 1),
                            skip_group_check=True,
                        )
                for j in range(4):
                    nc.vector.tensor_copy(kT_t[j][:, ssl], ps_k[j][:])

                # ---- q projection: qT[j, s-chunk] ----
                ps_q = [ps_main.tile([P, CHUNK], F32, name=f"psq{j}", tag=f"pm{j}")
                        for j in range(4)]
                for d in range(DT):
                    wq_d = wst_pool.tile([P, JL], F32R, name=f"wq{d}", tag="wst")
                    nc.sync.dma_start(out=wq_d[:], in_=wqT[d * P:(d + 1) * P, :])
                    for j in range(4):
                        nc.tensor.matmul(
                            ps_q[j][:], wq_d[:, j * P:(j + 1) * P], xt[d][:],
                            start=(d == 0), stop=(d == DT - 1),
                            skip_group_check=True,
                        )
                qt = []
                for j in range(4):
                    t_ = qt_pool.tile([P, CHUNK], F32R, name=f"qt{j}", tag=f"qt{j}")
                    nc.vector.tensor_copy(t_[:], ps_q[j][:])
                    qt.append(t_)

                # ---- v projection: v[t, j] for the 4 new t-tiles ----
                ps_v = [ps_main.tile([P, CHUNK], F32, name=f"psv{i}", tag=f"pm{i}")
                        for i in range(4)]
                for d in range(DT):
                    wv_d = wst_pool.tile([P, JL], F32R, name=f"wv{d}", tag="wst")
                    nc.sync.dma_start(out=wv_d[:], in_=wvT[d * P:(d + 1) * P, :])
                    for i in range(4):
                        nc.tensor.matmul(
                            ps_v[i][:], xt[d][:, i * P:(i + 1) * P], wv_d[:],
                            start=(d == 0), stop=(d == DT - 1),
                            skip_group_check=True,
                        )
                for i in range(4):
                    nc.vector.tensor_copy(v_t[4 * c + i][:], ps_v[i][:])

                # ---- attention for each head over t-tiles 0..4c+3 ----
                ots = []
                for h in range(NH):
                    T = 4 * c + 4
                    rs_acc = ps_rs.tile([1, CHUNK], F32, name="rsacc", tag="rs")
                    o_acc = ps_s.tile([P, CHUNK], F32, name="oacc", tag="ss")
                    exps = [None] * T

                    def emit_b(t):
                        nc.tensor.matmul(
                            rs_acc[:], ones[:], exps[t][:],
                            start=(t == 0), stop=(t == T - 1),
                            skip_group_check=True,
                        )
                        nc.tensor.matmul(
                            o_acc[:], v_t[t][:, h * P:(h + 1) * P], exps[t][:],
                            start=(t == 0), stop=(t == T - 1),
                            skip_group_check=True,
                        )

                    ngroups = T // 4
                    for g in range(ngroups):
                        for i in range(4):
                            t = 4 * g + i
                            ps = ps_s.tile([P, CHUNK], F32, name="pss", tag="ss")
                            nc.tensor.matmul(
                                ps[:], kT_t[h][:, t * P:(t + 1) * P], qt[h][:],
                                start=True, stop=True, skip_group_check=True,
                            )
                            e = exp_pool.tile([P, CHUNK], F32R, name="exp", tag="exp")
                            nc.scalar.activation(
                                e[:], ps[:], mybir.ActivationFunctionType.Exp,
                                scale=SCALE,
                            )
                            if t >= 4 * c:
                                off = 384 + s0 - t * P
                                nc.vector.tensor_mul(
                                    e[:], e[:], master[:, off:off + CHUNK])
                            exps[t] = e
                        if g >= 1:
                            for i in range(4):
                                emit_b(4 * (g - 1) + i)
                    for i in range(4):
                        emit_b(4 * (ngroups - 1) + i)

                    # normalize: reciprocal (PSUM->SBUF), broadcast, multiply
                    rs_sb = small_pool.tile([1, CHUNK], F32, name="rssb", tag="rssb")
                    nc.vector.reciprocal_approx_fast(out=rs_sb[:], in_=rs_acc[:])
                    rb = small_pool.tile([P, CHUNK], F32, name="rb", tag="rb")
                    nc.gpsimd.partition_broadcast(rb[:], rs_sb[:])
                    ot = ot_pool.tile([P, CHUNK], F32R, name="ot", tag="ot")
                    nc.vector.tensor_mul(ot[:], o_acc[:], rb[:])
                    ots.append(ot)

                # ---- wo partial: outT[j2, s-chunk] = sum_h woT_h^T @ ot_h ----
                for j2 in range(DT):
                    pw = ps_s.tile([P, CHUNK], F32, name="pw", tag="ss")
                    for h in range(NH):
                        nc.tensor.matmul(
                            pw[:], woT_t[h][:, j2 * P:(j2 + 1) * P], ots[h][:],
                            start=(h == 0), stop=(h == NH - 1),
                            skip_group_check=True,
                        )
                    ob = osb_pool.tile([P, CHUNK], F32, name="ob", tag="ob")
                    nc.vector.tensor_copy(ob[:], pw[:])
                    nc.scalar.dma_start(out=outT[j2 * P:(j2 + 1) * P, ssl],
                                        in_=ob[:])

    nc.compile()
    return nc


_NC_CACHE = None


def _get_nc():
    global _NC_CACHE
    if _NC_CACHE is None:
        _NC_CACHE = build_kernel()
    return _NC_CACHE


def make_in_maps(x, wq, wk, wv, wo):
    in_maps = []
    for core in range(8):
        b, g = core // 4, core % 4
        j0 = g * JL
        in_maps.append({
            "xT": np.ascontiguousarray(x[b].T).astype(np.float32, copy=False),
            "wqT": np.ascontiguousarray(wq[j0:j0 + JL, :].T),
            "wkT": np.ascontiguousarray(wk[j0:j0 + JL, :].T),
            "wvT": np.ascontiguousarray(wv[j0:j0 + JL, :].T),
            "woT": np.ascontiguousarray(wo[:, j0:j0 + JL].T),
        })
    return in_maps


def kernel(x, freqs_complex=None, mask=None, wq=None, wk=None, wv=None, wo=None,
           **_unused):
    x = np.asarray(x, dtype=np.float32)
    wq = np.asarray(wq, dtype=np.float32)
    wk = np.asarray(wk, dtype=np.float32)
    wv = np.asarray(wv, dtype=np.float32)
    wo = np.asarray(wo, dtype=np.float32)

    nc = _get_nc()
    in_maps = make_in_maps(x, wq, wk, wv, wo)
    res = run_bass_kernel_spmd(nc, in_maps, list(range(8)))

    out = np.zeros((B, S, D), dtype=np.float32)
    for core in range(8):
        out[core // 4] += res.results[core]["outT"].T
    return out
